# revision 6
# baseline (speedup 1.0000x reference)
"""Trainium2 Bass kernel: 2-layer LSTM (B=1024, T=512, H=256) + linear head.

Data-parallel across 8 NeuronCores: each core runs the full sequential scan
for a 128-row batch shard. Host-side work is marshaling only: sharding,
weight transposes/permutation, folding the day-embedding into layer-0 input
weights, and one-hot encoding the integer day column.

Device schedule (wavefront, one tick per timestep):
  tick t: [PE] transpose h1[t-2] | gates0[t] matmuls | gates1[t-1] matmuls
               (PSUM-bank interleaved, Whh1 terms last) | transpose h0[t]
          [ACT] cast h1T[t-2] | sigmoid0 | tanh g0 | tanh c0 | sigmoid1 | ...
          [DVE] cell/hidden updates + cast h0T[t]
Gates live in PSUM [B=128, 4H=1024] (two banks, matmuls N=512 float32r,
stationary = transposed state, moving = transposed weights). Biases ride
matmuls: layer 0 via the aug ones-row, layer 1 via a K=128 broadcast
matmul (e0 row-selector x full-row bias matrix). Gate columns are permuted
[i f g o] -> [i f o g] so one sigmoid instruction covers cols 0:768.
"""

import sys

import numpy as np

try:
    import concourse.bass as _probe  # noqa: F401
except ImportError:
    sys.path.insert(0, "/opt/trn_rl_repo")

B_FULL, T, D, H, P_OUT = 1024, 512, 64, 256, 14
N_CORES = 8
B = B_FULL // N_CORES  # 128 rows per core
G = 4 * H  # 1024 gate width
FA = 16  # augmented input rows: [val, onehot(day) x7, ones, pad x7]
CH = 64  # timesteps per aug SBUF chunk
NCH = T // CH

_PERM = np.concatenate(
    [np.arange(0, 512), np.arange(768, 1024), np.arange(512, 768)]
)

_MODULE = None
LAST_RESULTS = None


def _build_module():
    from contextlib import ExitStack

    import concourse.mybir as mybir
    from concourse import bacc
    from concourse.masks import make_identity
    from concourse.tile import TileContext

    f32 = mybir.dt.float32
    f32r = mybir.dt.float32r
    Sig = mybir.ActivationFunctionType.Sigmoid
    Tanh = mybir.ActivationFunctionType.Tanh

    nc = bacc.Bacc()
    aug_d = nc.dram_tensor("aug", [FA, T * B], f32r, kind="ExternalInput")
    z112_d = nc.dram_tensor("z112", [128 - FA, CH * B], f32r, kind="ExternalInput")
    w0t_d = nc.dram_tensor("w0t", [128, G], f32r, kind="ExternalInput")
    whh0t_d = nc.dram_tensor("whh0t", [H, G], f32r, kind="ExternalInput")
    wih1t_d = nc.dram_tensor("wih1t", [H, G], f32r, kind="ExternalInput")
    whh1t_d = nc.dram_tensor("whh1t", [H, G], f32r, kind="ExternalInput")
    e0_d = nc.dram_tensor("e0", [128, 128], f32r, kind="ExternalInput")
    b1f_d = nc.dram_tensor("b1f", [128, G], f32r, kind="ExternalInput")
    wlint_d = nc.dram_tensor("wlint", [H, P_OUT], f32r, kind="ExternalInput")
    blinf_d = nc.dram_tensor("blinf", [128, P_OUT], f32r, kind="ExternalInput")
    out_d = nc.dram_tensor("out", [B, P_OUT], f32, kind="ExternalOutput")

    with TileContext(nc) as tc, ExitStack() as ctx:
        consts = ctx.enter_context(tc.tile_pool(name="consts", bufs=1))
        h0Tp = ctx.enter_context(tc.tile_pool(name="h0Tp", bufs=3))
        h1Tp = ctx.enter_context(tc.tile_pool(name="h1Tp", bufs=3))
        c0p = ctx.enter_context(tc.tile_pool(name="c0p", bufs=2))
        c1p = ctx.enter_context(tc.tile_pool(name="c1p", bufs=2))
        acts = ctx.enter_context(tc.tile_pool(name="acts", bufs=2))
        g0pp = ctx.enter_context(tc.tile_pool(name="g0pp", bufs=1, space="PSUM"))
        g1pp = ctx.enter_context(tc.tile_pool(name="g1pp", bufs=2, space="PSUM"))
        hTps = ctx.enter_context(tc.tile_pool(name="hTps", bufs=2, space="PSUM"))

        # --- constants to SBUF ---
        w0t_sb = consts.tile([128, G], f32r, tag="w0t")
        nc.sync.dma_start(w0t_sb, w0t_d[:, :])
        whh0t_sb = consts.tile([128, 2 * G], f32r, tag="whh0t")
        wih1t_sb = consts.tile([128, 2 * G], f32r, tag="wih1t")
        whh1t_sb = consts.tile([128, 2 * G], f32r, tag="whh1t")
        for k in range(2):
            nc.sync.dma_start(
                whh0t_sb[:, k * G : (k + 1) * G], whh0t_d[k * 128 : (k + 1) * 128, :]
            )
            nc.sync.dma_start(
                wih1t_sb[:, k * G : (k + 1) * G], wih1t_d[k * 128 : (k + 1) * 128, :]
            )
            nc.sync.dma_start(
                whh1t_sb[:, k * G : (k + 1) * G], whh1t_d[k * 128 : (k + 1) * 128, :]
            )
        e0_sb = consts.tile([128, 128], f32r, tag="e0")
        nc.sync.dma_start(e0_sb, e0_d[:, :])
        b1f_sb = consts.tile([128, G], f32r, tag="b1f")
        nc.sync.dma_start(b1f_sb, b1f_d[:, :])
        wlint_sb = consts.tile([128, 2 * P_OUT], f32r, tag="wlint")
        for k in range(2):
            nc.sync.dma_start(
                wlint_sb[:, k * P_OUT : (k + 1) * P_OUT],
                wlint_d[k * 128 : (k + 1) * 128, :],
            )
        blinf_sb = consts.tile([128, P_OUT], f32r, tag="blinf")
        nc.sync.dma_start(blinf_sb, blinf_d[:, :])
        ident = consts.tile([128, 128], f32, tag="ident")
        make_identity(nc, ident)

        # Two persistent aug buffers (manual double-buffer). Rows FA:128 are
        # zeroed once so the aug matmul can run with K=128.
        aug_bufs = []
        for i in range(2):
            ab = consts.tile([128, CH * B], f32r, tag=f"augbuf{i}", name=f"augbuf{i}")
            nc.sync.dma_start(ab[FA:128, :], z112_d[:, :])
            aug_bufs.append(ab)

        def load_chunk(chi):
            nc.sync.dma_start(
                aug_bufs[chi % 2][0:FA, :],
                aug_d[:, chi * CH * B : (chi + 1) * CH * B],
            )

        load_chunk(0)
        load_chunk(1)

        mm = nc.tensor.matmul

        # state handles indexed by step (python refs; tiles come from pools)
        h0T = [None] * T
        h1T = [None] * T
        c0 = [None] * T
        c1 = [None] * T
        h0n = [None] * T
        h1n = [None] * T
        sig = [[None] * T, [None] * T]
        gt = [[None] * T, [None] * T]
        g0ps = [None] * T
        g1ps = [None] * T
        h1tps = [None] * T

        def emit_g0_mms(t):
            chi = t // CH
            if t % CH == 0:
                if chi + 2 < NCH:
                    load_chunk(chi + 2)
            aug_sl = aug_bufs[chi % 2][:, (t % CH) * B : (t % CH + 1) * B]
            g0 = g0pp.tile([B, G], f32, tag="g0", name=f"g0_{t}")
            g0ps[t] = g0
            bk = [slice(0, 512), slice(512, 1024)]
            if t == 0:
                for nb in range(2):
                    mm(g0[:, bk[nb]], aug_sl, w0t_sb[:, bk[nb]], start=True, stop=True)
                return
            hp = h0T[t - 1]
            for nb in range(2):
                mm(g0[:, bk[nb]], aug_sl, w0t_sb[:, bk[nb]], start=True, stop=False)
            for k in range(2):
                for nb in range(2):
                    mm(
                        g0[:, bk[nb]],
                        hp[:, k * 128 : (k + 1) * 128],
                        whh0t_sb[:, k * G + nb * 512 : k * G + (nb + 1) * 512],
                        start=False,
                        stop=(k == 1),
                    )

        def emit_g1_mms(t):
            g1 = g1pp.tile([B, G], f32, tag="g1", name=f"g1_{t}")
            g1ps[t] = g1
            bk = [slice(0, 512), slice(512, 1024)]
            for nb in range(2):
                mm(g1[:, bk[nb]], e0_sb, b1f_sb[:, bk[nb]], start=True, stop=False)
            hp = h0T[t]
            for k in range(2):
                for nb in range(2):
                    mm(
                        g1[:, bk[nb]],
                        hp[:, k * 128 : (k + 1) * 128],
                        wih1t_sb[:, k * G + nb * 512 : k * G + (nb + 1) * 512],
                        start=False,
                        stop=(t == 0 and k == 1),
                    )
            if t > 0:
                hq = h1T[t - 1]
                for k in range(2):
                    for nb in range(2):
                        mm(
                            g1[:, bk[nb]],
                            hq[:, k * 128 : (k + 1) * 128],
                            whh1t_sb[:, k * G + nb * 512 : k * G + (nb + 1) * 512],
                            start=False,
                            stop=(k == 1),
                        )

        def emit_chain(layer, t):
            gps = g0ps[t] if layer == 0 else g1ps[t]
            cp = c0p if layer == 0 else c1p
            cl = c0 if layer == 0 else c1
            hn = h0n if layer == 0 else h1n
            s = acts.tile([B, 3 * H], f32, tag=f"sig{layer}", name=f"sig{layer}_{t}")
            sig[layer][t] = s
            nc.scalar.activation(s, gps[:, 0 : 3 * H], Sig)
            g = acts.tile([B, H], f32, tag=f"gt{layer}", name=f"gt{layer}_{t}")
            gt[layer][t] = g
            nc.scalar.activation(g, gps[:, 3 * H : G], Tanh)
            cn = cp.tile([B, H], f32, tag=f"c{layer}", name=f"c{layer}_{t}")
            if t == 0:
                nc.vector.tensor_mul(cn, s[:, 0:H], g)
            else:
                ig = acts.tile([B, H], f32, tag=f"ig{layer}", name=f"ig{layer}_{t}")
                nc.vector.tensor_mul(ig, s[:, 0:H], g)
                fc = acts.tile([B, H], f32, tag=f"fc{layer}", name=f"fc{layer}_{t}")
                nc.vector.tensor_mul(fc, s[:, H : 2 * H], cl[t - 1])
                nc.vector.tensor_add(cn, ig, fc)
            cl[t] = cn
            tcx = acts.tile([B, H], f32, tag=f"tc{layer}", name=f"tc{layer}_{t}")
            nc.scalar.activation(tcx, cn, Tanh)
            h = acts.tile([B, H], f32, tag=f"hn{layer}", name=f"hn{layer}_{t}")
            nc.vector.tensor_mul(h, s[:, 2 * H : 3 * H], tcx)
            hn[t] = h

        def emit_h0_transp(t):
            ps = hTps.tile([128, 512], f32, tag="htp", name=f"h0tp_{t}")
            nc.tensor.transpose(ps[:, 0:128], h0n[t][:, 0:128], ident)
            nc.tensor.transpose(ps[:, 256:384], h0n[t][:, 128:256], ident)
            hsb = h0Tp.tile([128, H], f32r, tag="h0T", name=f"h0T_{t}")
            nc.vector.tensor_copy(
                hsb.rearrange("p (b c) -> p b c", b=2),
                ps.rearrange("p (b c) -> p b c", b=2)[:, :, 0:128],
            )
            h0T[t] = hsb

        def emit_h1_transp(t):
            ps = hTps.tile([128, 512], f32, tag="htp", name=f"h1tp_{t}")
            nc.tensor.transpose(ps[:, 0:128], h1n[t][:, 0:128], ident)
            nc.tensor.transpose(ps[:, 256:384], h1n[t][:, 128:256], ident)
            h1tps[t] = ps

        def emit_h1_cast(t):
            hsb = h1Tp.tile([128, H], f32r, tag="h1T", name=f"h1T_{t}")
            nc.vector.tensor_copy(
                hsb.rearrange("p (b c) -> p b c", b=2),
                h1tps[t].rearrange("p (b c) -> p b c", b=2)[:, :, 0:128],
            )
            h1T[t] = hsb

        for tau in range(T + 2):
            if tau >= 2:
                emit_h1_transp(tau - 2)  # PE slot 0: h1n[tau-2] long ready
                emit_h1_cast(tau - 2)  # ACT, in its idle window at tick start
            if tau < T:
                emit_g0_mms(tau)
            if 1 <= tau <= T:
                emit_g1_mms(tau - 1)
            if tau < T:
                emit_chain(0, tau)
                emit_h0_transp(tau)
            if 1 <= tau <= T:
                emit_chain(1, tau - 1)

        # ------------- final linear: out = h1[T-1] @ Wlin.T + blin -------------
        outp = hTps.tile([B, P_OUT], f32, tag="htp", name="outp")
        mm(outp, e0_sb, blinf_sb, start=True, stop=False)
        hl = h1T[T - 1]
        for k in range(2):
            mm(
                outp,
                hl[:, k * 128 : (k + 1) * 128],
                wlint_sb[:, k * P_OUT : (k + 1) * P_OUT],
                start=False,
                stop=(k == 1),
            )
        out_sb = consts.tile([B, P_OUT], f32, tag="outsb")
        nc.vector.tensor_copy(out_sb, outp)
        nc.sync.dma_start(out_d[:, :], out_sb)

    nc.finalize()
    return nc


def _get_module():
    global _MODULE
    if _MODULE is None:
        _MODULE = _build_module()
    return _MODULE


def kernel(**inputs):
    global LAST_RESULTS
    from concourse.bass_utils import run_bass_kernel_spmd

    f = lambda a: np.ascontiguousarray(np.asarray(a), dtype=np.float32)
    x = f(inputs["x"])
    emb = f(inputs["emb"])
    Wih0, Whh0 = f(inputs["Wih0"]), f(inputs["Whh0"])
    bih0, bhh0 = f(inputs["bih0"]), f(inputs["bhh0"])
    Wih1, Whh1 = f(inputs["Wih1"]), f(inputs["Whh1"])
    bih1, bhh1 = f(inputs["bih1"]), f(inputs["bhh1"])
    Wlin, blin = f(inputs["Wlin"]), f(inputs["blin"])

    # Fold embedding + biases into layer-0 input weights.
    w_val = Wih0[:, 0:1]  # [G, 1]
    M0 = Wih0[:, 1 : 1 + D] @ emb.T  # [G, 7]
    b0 = (bih0 + bhh0)[:, None]  # [G, 1]
    W0aug = np.concatenate(
        [w_val, M0, b0, np.zeros((G, 128 - 9), np.float32)], axis=1
    )  # [G, 128]

    w0t = np.ascontiguousarray(W0aug[_PERM].T)  # [128, G]
    whh0t = np.ascontiguousarray(Whh0[_PERM].T)  # [H, G]
    wih1t = np.ascontiguousarray(Wih1[_PERM].T)
    whh1t = np.ascontiguousarray(Whh1[_PERM].T)
    b1f = np.zeros((128, G), np.float32)
    b1f[0] = (bih1 + bhh1)[_PERM]
    e0 = np.zeros((128, 128), np.float32)
    e0[0] = 1.0
    wlint = np.ascontiguousarray(Wlin.T)  # [H, P_OUT]
    blinf = np.zeros((128, P_OUT), np.float32)
    blinf[0] = blin
    z112 = np.zeros((128 - FA, CH * B), np.float32)

    val = x[:, :, 0]  # [B_FULL, T]
    day = x[:, :, 1].astype(np.int32)  # [B_FULL, T]

    in_maps = []
    for c in range(N_CORES):
        sl = slice(c * B, (c + 1) * B)
        aug = np.zeros((FA, T, B), np.float32)
        aug[0] = val[sl].T
        dT = day[sl].T  # [T, B]
        for d in range(7):
            aug[1 + d] = dT == d
        aug[8] = 1.0
        in_maps.append(
            {
                "aug": np.ascontiguousarray(aug.reshape(FA, T * B)),
                "z112": z112,
                "w0t": w0t,
                "whh0t": whh0t,
                "wih1t": wih1t,
                "whh1t": whh1t,
                "e0": e0,
                "b1f": b1f,
                "wlint": wlint,
                "blinf": blinf,
            }
        )

    res = run_bass_kernel_spmd(_get_module(), in_maps, core_ids=list(range(N_CORES)))
    LAST_RESULTS = res
    out = np.concatenate([r["out"] for r in res.results], axis=0)
    return np.ascontiguousarray(out, dtype=np.float32)


# revision 8
# speedup vs baseline: 1.1914x; 1.1914x over previous
"""Trainium2 Bass kernel: 2-layer LSTM (B=1024, T=512, H=256) + linear head.

Data-parallel across 8 NeuronCores: each core runs the full sequential scan
for a 128-row batch shard. Host-side work is marshaling only: sharding,
weight transposes/permutation, folding the day-embedding into layer-0 input
weights, and one-hot encoding the integer day column.

Device schedule (wavefront, one tick per timestep):
  tick t: [PE] transpose h1[t-2] | gates0[t] matmuls | gates1[t-1] matmuls
               (PSUM-bank interleaved, Whh1 terms last) | transpose h0[t]
          [ACT] cast h1T[t-2] | sigmoid0 | tanh g0 | tanh c0 | sigmoid1 | ...
          [DVE] cell/hidden updates + cast h0T[t]
Gates live in PSUM [B=128, 4H=1024] (two banks, matmuls N=512 float32r,
stationary = transposed state, moving = transposed weights). Biases ride
matmuls: layer 0 via the aug ones-row, layer 1 via a K=128 broadcast
matmul (e0 row-selector x full-row bias matrix). Gate columns are permuted
[i f g o] -> [i f o g] so one sigmoid instruction covers cols 0:768.
"""

import sys

import numpy as np

try:
    import concourse.bass as _probe  # noqa: F401
except ImportError:
    sys.path.insert(0, "/opt/trn_rl_repo")

B_FULL, T, D, H, P_OUT = 1024, 512, 64, 256, 14
N_CORES = 8
B = B_FULL // N_CORES  # 128 rows per core
G = 4 * H  # 1024 gate width
FA = 16  # augmented input rows: [val, onehot(day) x7, ones, pad x7]
CH = 64  # timesteps per aug SBUF chunk
NCH = T // CH

_PERM = np.concatenate(
    [np.arange(0, 512), np.arange(768, 1024), np.arange(512, 768)]
)

_MODULE = None
LAST_RESULTS = None


def _build_module():
    from contextlib import ExitStack

    import concourse.mybir as mybir
    from concourse import bacc
    from concourse.masks import make_identity
    from concourse.tile import TileContext

    f32 = mybir.dt.float32
    f32r = mybir.dt.float32r
    bf16 = mybir.dt.bfloat16
    Sig = mybir.ActivationFunctionType.Sigmoid
    Tanh = mybir.ActivationFunctionType.Tanh

    nc = bacc.Bacc()
    aug_d = nc.dram_tensor("aug", [FA, T * B], f32r, kind="ExternalInput")
    z112_d = nc.dram_tensor("z112", [128 - FA, CH * B], f32r, kind="ExternalInput")
    w0t_d = nc.dram_tensor("w0t", [128, G], f32r, kind="ExternalInput")
    whh0t_d = nc.dram_tensor("whh0t", [H, G], f32r, kind="ExternalInput")
    wih1t_d = nc.dram_tensor("wih1t", [H, G], f32r, kind="ExternalInput")
    whh1t_d = nc.dram_tensor("whh1t", [H, G], f32r, kind="ExternalInput")
    e0_d = nc.dram_tensor("e0", [128, 128], f32r, kind="ExternalInput")
    b1f_d = nc.dram_tensor("b1f", [128, G], f32r, kind="ExternalInput")
    wlint_d = nc.dram_tensor("wlint", [H, P_OUT], f32r, kind="ExternalInput")
    blinf_d = nc.dram_tensor("blinf", [128, P_OUT], f32r, kind="ExternalInput")
    out_d = nc.dram_tensor("out", [B, P_OUT], f32, kind="ExternalOutput")

    with TileContext(nc) as tc, ExitStack() as ctx:
        consts = ctx.enter_context(tc.tile_pool(name="consts", bufs=1))
        h0Tp = ctx.enter_context(tc.tile_pool(name="h0Tp", bufs=3))
        h1Tp = ctx.enter_context(tc.tile_pool(name="h1Tp", bufs=3))
        c0p = ctx.enter_context(tc.tile_pool(name="c0p", bufs=2))
        c1p = ctx.enter_context(tc.tile_pool(name="c1p", bufs=2))
        acts = ctx.enter_context(tc.tile_pool(name="acts", bufs=2))
        g0pp = ctx.enter_context(tc.tile_pool(name="g0pp", bufs=1, space="PSUM"))
        g1pp = ctx.enter_context(tc.tile_pool(name="g1pp", bufs=2, space="PSUM"))
        hTps = ctx.enter_context(tc.tile_pool(name="hTps", bufs=2, space="PSUM"))

        # --- constants to SBUF ---
        w0t_sb = consts.tile([128, G], f32r, tag="w0t")
        nc.sync.dma_start(w0t_sb, w0t_d[:, :])
        whh0t_sb = consts.tile([128, 2 * G], f32r, tag="whh0t")
        wih1t_sb = consts.tile([128, 2 * G], f32r, tag="wih1t")
        whh1t_sb = consts.tile([128, 2 * G], f32r, tag="whh1t")
        for k in range(2):
            nc.sync.dma_start(
                whh0t_sb[:, k * G : (k + 1) * G], whh0t_d[k * 128 : (k + 1) * 128, :]
            )
            nc.sync.dma_start(
                wih1t_sb[:, k * G : (k + 1) * G], wih1t_d[k * 128 : (k + 1) * 128, :]
            )
            nc.sync.dma_start(
                whh1t_sb[:, k * G : (k + 1) * G], whh1t_d[k * 128 : (k + 1) * 128, :]
            )
        e0_sb = consts.tile([128, 128], f32r, tag="e0")
        nc.sync.dma_start(e0_sb, e0_d[:, :])
        b1f_sb = consts.tile([128, G], f32r, tag="b1f")
        nc.sync.dma_start(b1f_sb, b1f_d[:, :])
        wlint_sb = consts.tile([128, 2 * P_OUT], f32r, tag="wlint")
        for k in range(2):
            nc.sync.dma_start(
                wlint_sb[:, k * P_OUT : (k + 1) * P_OUT],
                wlint_d[k * 128 : (k + 1) * 128, :],
            )
        blinf_sb = consts.tile([128, P_OUT], f32r, tag="blinf")
        nc.sync.dma_start(blinf_sb, blinf_d[:, :])
        ident = consts.tile([128, 128], f32, tag="ident")
        make_identity(nc, ident)

        # Two persistent aug buffers (manual double-buffer). Rows FA:128 are
        # zeroed once so the aug matmul can run with K=128.
        aug_bufs = []
        for i in range(2):
            ab = consts.tile([128, CH * B], f32r, tag=f"augbuf{i}", name=f"augbuf{i}")
            nc.sync.dma_start(ab[FA:128, :], z112_d[:, :])
            aug_bufs.append(ab)

        def load_chunk(chi):
            nc.sync.dma_start(
                aug_bufs[chi % 2][0:FA, :],
                aug_d[:, chi * CH * B : (chi + 1) * CH * B],
            )

        load_chunk(0)
        load_chunk(1)

        mm = nc.tensor.matmul

        # state handles indexed by step (python refs; tiles come from pools)
        h0T = [None] * T
        h1T = [None] * T
        c0 = [None] * T
        c1 = [None] * T
        h0n = [None] * T
        h1n = [None] * T
        sig = [[None] * T, [None] * T]
        gt = [[None] * T, [None] * T]
        g0ps = [None] * T
        g1ps = [None] * T
        h1tps = [None] * T

        def emit_g0_mms(t):
            chi = t // CH
            if t % CH == 0:
                if chi + 2 < NCH:
                    load_chunk(chi + 2)
            aug_sl = aug_bufs[chi % 2][:, (t % CH) * B : (t % CH + 1) * B]
            g0 = g0pp.tile([B, G], f32, tag="g0", name=f"g0_{t}")
            g0ps[t] = g0
            bk = [slice(0, 512), slice(512, 1024)]
            if t == 0:
                for nb in range(2):
                    mm(g0[:, bk[nb]], aug_sl, w0t_sb[:, bk[nb]], start=True, stop=True)
                return
            hp = h0T[t - 1]
            for nb in range(2):
                mm(g0[:, bk[nb]], aug_sl, w0t_sb[:, bk[nb]], start=True, stop=False)
            for k in range(2):
                for nb in range(2):
                    mm(
                        g0[:, bk[nb]],
                        hp[:, k * 128 : (k + 1) * 128],
                        whh0t_sb[:, k * G + nb * 512 : k * G + (nb + 1) * 512],
                        start=False,
                        stop=(k == 1),
                    )

        def emit_g1_bias_ih1(t):
            g1 = g1pp.tile([B, G], f32, tag="g1", name=f"g1_{t}")
            g1ps[t] = g1
            bk = [slice(0, 512), slice(512, 1024)]
            for nb in range(2):
                mm(g1[:, bk[nb]], e0_sb, b1f_sb[:, bk[nb]], start=True, stop=False)
            hp = h0T[t]
            for k in range(2):
                for nb in range(2):
                    mm(
                        g1[:, bk[nb]],
                        hp[:, k * 128 : (k + 1) * 128],
                        wih1t_sb[:, k * G + nb * 512 : k * G + (nb + 1) * 512],
                        start=False,
                        stop=(t == 0 and k == 1),
                    )

        def emit_g1_hh1(t):
            g1 = g1ps[t]
            bk = [slice(0, 512), slice(512, 1024)]
            hq = h1T[t - 1]
            for k in range(2):
                for nb in range(2):
                    mm(
                        g1[:, bk[nb]],
                        hq[:, k * 128 : (k + 1) * 128],
                        whh1t_sb[:, k * G + nb * 512 : k * G + (nb + 1) * 512],
                        start=False,
                        stop=(k == 1),
                    )

        sigo0 = [None] * T

        def emit_chain_a0(t):
            gps = g0ps[t]
            s = acts.tile([B, 2 * H], bf16, tag="sigif0", name=f"sigif0_{t}")
            sig[0][t] = s
            nc.scalar.activation(s, gps[:, 0 : 2 * H], Sig)
            g = acts.tile([B, H], bf16, tag="gt0", name=f"gt0_{t}")
            gt[0][t] = g
            nc.scalar.activation(g, gps[:, 3 * H : G], Tanh)
            so = acts.tile([B, H], bf16, tag="sigo0", name=f"sigo0_{t}")
            sigo0[t] = so
            nc.scalar.activation(so, gps[:, 2 * H : 3 * H], Sig)

        def emit_chain_a1(t):
            gps = g1ps[t]
            s = acts.tile([B, 3 * H], bf16, tag="sig1", name=f"sig1_{t}")
            sig[1][t] = s
            nc.scalar.activation(s, gps[:, 0 : 3 * H], Sig)
            g = acts.tile([B, H], bf16, tag="gt1", name=f"gt1_{t}")
            gt[1][t] = g
            nc.scalar.activation(g, gps[:, 3 * H : G], Tanh)

        def emit_chain_b1(layer, t):
            cp = c0p if layer == 0 else c1p
            cl = c0 if layer == 0 else c1
            s = sig[layer][t]
            g = gt[layer][t]
            cn = cp.tile([B, H], f32, tag=f"c{layer}", name=f"c{layer}_{t}")
            if t == 0:
                nc.vector.tensor_mul(cn, s[:, 0:H], g)
            else:
                ig = acts.tile([B, H], bf16, tag=f"ig{layer}", name=f"ig{layer}_{t}")
                nc.vector.tensor_mul(ig, s[:, 0:H], g)
                fc = acts.tile([B, H], bf16, tag=f"fc{layer}", name=f"fc{layer}_{t}")
                nc.vector.tensor_mul(fc, s[:, H : 2 * H], cl[t - 1])
                nc.vector.tensor_add(cn, ig, fc)
            cl[t] = cn
            tcx = acts.tile([B, H], bf16, tag=f"tc{layer}", name=f"tc{layer}_{t}")
            nc.scalar.activation(tcx, cn, Tanh)
            return tcx

        def emit_chain_b2(layer, t, tcx):
            hn = h0n if layer == 0 else h1n
            so = sigo0[t] if layer == 0 else sig[1][t][:, 2 * H : 3 * H]
            h = acts.tile([B, H], f32, tag=f"hn{layer}", name=f"hn{layer}_{t}")
            nc.vector.tensor_mul(h, so, tcx)
            hn[t] = h

        def emit_h0_transp(t):
            ps = hTps.tile([128, 512], f32, tag="htp", name=f"h0tp_{t}")
            nc.tensor.transpose(ps[:, 0:128], h0n[t][:, 0:128], ident)
            nc.tensor.transpose(ps[:, 256:384], h0n[t][:, 128:256], ident)
            hsb = h0Tp.tile([128, H], f32r, tag="h0T", name=f"h0T_{t}")
            nc.vector.tensor_copy(
                hsb.rearrange("p (b c) -> p b c", b=2),
                ps.rearrange("p (b c) -> p b c", b=2)[:, :, 0:128],
            )
            h0T[t] = hsb

        def emit_h1_transp(t):
            ps = hTps.tile([128, 512], f32, tag="htp", name=f"h1tp_{t}")
            nc.tensor.transpose(ps[:, 0:128], h1n[t][:, 0:128], ident)
            nc.tensor.transpose(ps[:, 256:384], h1n[t][:, 128:256], ident)
            h1tps[t] = ps

        def emit_h1_cast(t):
            hsb = h1Tp.tile([128, H], f32r, tag="h1T", name=f"h1T_{t}")
            nc.vector.tensor_copy(
                hsb.rearrange("p (b c) -> p b c", b=2),
                h1tps[t].rearrange("p (b c) -> p b c", b=2)[:, :, 0:128],
            )
            h1T[t] = hsb

        for tau in range(T + 2):
            if tau < T:
                emit_g0_mms(tau)
                emit_chain_a0(tau)
                tc0x = emit_chain_b1(0, tau)
            else:
                tc0x = None
            if 1 <= tau <= T:
                emit_g1_bias_ih1(tau - 1)
            if tau >= 2 and tau - 2 < T:
                emit_h1_transp(tau - 2)
                emit_h1_cast(tau - 2)
            if 2 <= tau <= T:
                emit_g1_hh1(tau - 1)
            if tc0x is not None:
                emit_chain_b2(0, tau, tc0x)
                emit_h0_transp(tau)
            if 1 <= tau <= T:
                emit_chain_a1(tau - 1)
                tc1x = emit_chain_b1(1, tau - 1)
                emit_chain_b2(1, tau - 1, tc1x)

        # ------------- final linear: out = h1[T-1] @ Wlin.T + blin -------------
        outp = hTps.tile([B, P_OUT], f32, tag="htp", name="outp")
        mm(outp, e0_sb, blinf_sb, start=True, stop=False)
        hl = h1T[T - 1]
        for k in range(2):
            mm(
                outp,
                hl[:, k * 128 : (k + 1) * 128],
                wlint_sb[:, k * P_OUT : (k + 1) * P_OUT],
                start=False,
                stop=(k == 1),
            )
        out_sb = consts.tile([B, P_OUT], f32, tag="outsb")
        nc.vector.tensor_copy(out_sb, outp)
        nc.sync.dma_start(out_d[:, :], out_sb)

    nc.finalize()
    return nc


def _get_module():
    global _MODULE
    if _MODULE is None:
        _MODULE = _build_module()
    return _MODULE


def kernel(**inputs):
    global LAST_RESULTS
    from concourse.bass_utils import run_bass_kernel_spmd

    f = lambda a: np.ascontiguousarray(np.asarray(a), dtype=np.float32)
    x = f(inputs["x"])
    emb = f(inputs["emb"])
    Wih0, Whh0 = f(inputs["Wih0"]), f(inputs["Whh0"])
    bih0, bhh0 = f(inputs["bih0"]), f(inputs["bhh0"])
    Wih1, Whh1 = f(inputs["Wih1"]), f(inputs["Whh1"])
    bih1, bhh1 = f(inputs["bih1"]), f(inputs["bhh1"])
    Wlin, blin = f(inputs["Wlin"]), f(inputs["blin"])

    # Fold embedding + biases into layer-0 input weights.
    w_val = Wih0[:, 0:1]  # [G, 1]
    M0 = Wih0[:, 1 : 1 + D] @ emb.T  # [G, 7]
    b0 = (bih0 + bhh0)[:, None]  # [G, 1]
    W0aug = np.concatenate(
        [w_val, M0, b0, np.zeros((G, 128 - 9), np.float32)], axis=1
    )  # [G, 128]

    w0t = np.ascontiguousarray(W0aug[_PERM].T)  # [128, G]
    whh0t = np.ascontiguousarray(Whh0[_PERM].T)  # [H, G]
    wih1t = np.ascontiguousarray(Wih1[_PERM].T)
    whh1t = np.ascontiguousarray(Whh1[_PERM].T)
    b1f = np.zeros((128, G), np.float32)
    b1f[0] = (bih1 + bhh1)[_PERM]
    e0 = np.zeros((128, 128), np.float32)
    e0[0] = 1.0
    wlint = np.ascontiguousarray(Wlin.T)  # [H, P_OUT]
    blinf = np.zeros((128, P_OUT), np.float32)
    blinf[0] = blin
    z112 = np.zeros((128 - FA, CH * B), np.float32)

    val = x[:, :, 0]  # [B_FULL, T]
    day = x[:, :, 1].astype(np.int32)  # [B_FULL, T]

    in_maps = []
    for c in range(N_CORES):
        sl = slice(c * B, (c + 1) * B)
        aug = np.zeros((FA, T, B), np.float32)
        aug[0] = val[sl].T
        dT = day[sl].T  # [T, B]
        for d in range(7):
            aug[1 + d] = dT == d
        aug[8] = 1.0
        in_maps.append(
            {
                "aug": np.ascontiguousarray(aug.reshape(FA, T * B)),
                "z112": z112,
                "w0t": w0t,
                "whh0t": whh0t,
                "wih1t": wih1t,
                "whh1t": whh1t,
                "e0": e0,
                "b1f": b1f,
                "wlint": wlint,
                "blinf": blinf,
            }
        )

    res = run_bass_kernel_spmd(_get_module(), in_maps, core_ids=list(range(N_CORES)))
    LAST_RESULTS = res
    out = np.concatenate([r["out"] for r in res.results], axis=0)
    return np.ascontiguousarray(out, dtype=np.float32)
